# revision 30
# baseline (speedup 1.0000x reference)
"""GAT message-passing kernel for TRN2 (8-core SPMD).

Math (heads h, nodes n):
  t[n,h,:] = x[n] @ Ws[h].T            (t-space features, 64 per head)
  Ar[n,h]  = x[n] @ war[:,h]           (war = Ws[h].T @ a_r[h], folded weights)
  u        = exp(Ar)
  out[i, h*64:h*64+64] = elu( sum_{e:src=i} u[dst,h]*t[dst,h,:] / sum u[dst,h] )

Sharding: src-range per core. Phase 1 builds the Y table
[t~ h0-3 (256) | u h0-3 (4) | pad | t~ h4-7 | u h4-7 | pad] for all nodes
(replicated on every core, in core-private DRAM, split in two halves so
gather indices fit int16). Phase 2 gathers per-edge Y rows (edges sorted by
(window, half), padded to 128-edge blocks), builds a one-hot S on DVE, and
segment-sums via PE matmul into PSUM per 128-node window; then
normalize + elu + store.

Perf notes (v3):
- Phase 1 batches 8 node-tiles per DMA: x-loads on SyncE (2KB runs), full
  1280B-row Y-writes on ScalarE (fully contiguous destination region) so
  HWDGE issue time and descriptor overhead stay off the critical path.
- Gathers fetch only the used 544-col span of each 640-col row
  (elem 1088B, stride 1280B): the bass-level elem%256 assert is bypassed
  via direct InstDMAGatherAnt construction — the Q7 kernel computes byte
  lengths directly; only the row *stride* is encoded in 256B units.
- Phase 2 splits each (window, half) gather into <=9-block calls and
  round-robins the 4 SWDGE queues (concurrent desc-gen on 4 Q7 core
  pairs). Deep pools (gather bufs=10, psum 4+4 full banks) keep 4 windows
  in flight.
- idx/meta/iota load before phase 1; the first 4 windows issue their
  half-0 gathers before any half-1 gather so y0-table gathers overlap the
  y1-build tail of phase 1 (a half-1 call earlier would head-of-line-block
  the in-order Pool queue on the y1 fence).
"""

import math
import numpy as np
from contextlib import ExitStack

import concourse.bass as bass
import concourse.bacc as bacc
import concourse.mybir as mybir
from concourse.tile import TileContext
from concourse.tile import add_dep_helper

F32 = mybir.dt.float32
BF16 = mybir.dt.bfloat16
FP8 = mybir.dt.float8e4
I16 = mybir.dt.int16

P = 128
IN_FEAT = 256
HEADS = 8
OUT = 64
TD = HEADS * OUT  # 512
GW = 260   # matmul N per group: 4 heads x 64 + 4 u columns
GS = 272   # group stride in the Y row (544B, 32B-aligned for PE rhs)
YS = 2 * GS  # gathered span per row: 544 cols = 1088B
YW = 640   # DRAM row stride: 1280B (multiple of 256B, required by gather)
TB = 8     # phase-1 node-tiles per DMA batch


def dma_gather_span(eng, out_ap, in_ap, idxs_ap, num_idxs, num_idxs_reg,
                    elem_size, elem_step, queue_num):
    """nc.gpsimd.dma_gather minus the elem_size%256 restriction.

    The Q7 dma_gather kernel computes descriptor byte lengths as
    elem_size*dtype_size directly (any value works); only the row *stride*
    is ISA-encoded in 256B units. bass's helper asserts elem%256 for the
    transpose path and applies it unconditionally, so build the
    instruction with the same lowering calls the helper makes.
    """
    eng._assert_queue_num(queue_num)
    assert idxs_ap.dtype == mybir.dt.int16
    assert in_ap.dtype == out_ap.dtype
    dt_size = mybir.dt.size(in_ap.dtype)
    stride_bytes = elem_step * dt_size
    assert stride_bytes % 256 == 0 and stride_bytes // 256 < 256
    assert (elem_size * dt_size) % 32 == 0
    assert in_ap.ap[0][0] == elem_step
    assert in_ap.ap[-1][1] == out_ap.ap[-1][1] == elem_size
    _in_ap = eng.lower_ap_dma(in_ap, for_custom_bir_dma=True)
    _idxs_ap = eng.lower_ap(idxs_ap)
    _out_ap = eng.lower_ap(out_ap)
    return eng.add_instruction(
        mybir.InstDMAGatherAnt(
            name=eng.bass.get_next_instruction_name(),
            ins=[*_in_ap, _idxs_ap,
                 eng.lower_val_access(eng.to_reg(num_idxs_reg))],
            outs=[_out_ap],
            transpose=False,
            num_idxs=num_idxs,
            elem_size=elem_size,
            stride_bytes_256=stride_bytes // 256,
            gen_mode=0,
            single_packet=False,
            queue_num=queue_num,
            sbuf_tokens_per_rank=0,
            sbuf_free_dim_per_rank=0,
            sbuf_free_dim_pad_per_rank=0,
            sbuf_byte_offset=0,
        )
    )


class Config:
    def __init__(self, n_nodes, src, dst, n_cores=8, ch_max=9, warm_windows=4):
        self.n_cores = n_cores
        self.dt_y = BF16
        self.dt_meta = BF16
        self.dt_s = FP8

        self.n_nodes = n_nodes
        # nodes per core, multiple of 128
        self.w_per_core = math.ceil(n_nodes / (n_cores * P))
        self.npc = self.w_per_core * P
        self.n_pad = self.npc * n_cores          # padded node count (Y rows)
        self.x_tiles = self.n_pad // P
        # half split for int16 gather indices
        h0_tiles = self.x_tiles // 2
        assert h0_tiles * P < 32768 and (self.x_tiles - h0_tiles) * P < 32768
        self.h0_tiles = h0_tiles
        self.h0_rows = h0_tiles * P
        self.h1_rows = (self.x_tiles - h0_tiles) * P

        # ---- per-core edge grouping (by window, dst-half) ----
        W = self.w_per_core
        src = np.asarray(src, dtype=np.int64)
        dst = np.asarray(dst, dtype=np.int64)
        core = src // self.npc
        w = (src % self.npc) // P
        lsrc = src % P
        half = (dst >= self.h0_rows).astype(np.int64)
        lidx = dst - self.h0_rows * half

        counts = np.zeros((n_cores, W, 2), dtype=np.int64)
        np.add.at(counts, (core, w, half), 1)
        cap = counts.max(axis=0)  # [W, 2] max over cores
        self.cap_blocks = np.ceil(cap / P).astype(np.int64)  # C[w][h]
        self.tot_blocks = int(self.cap_blocks.sum())
        self.tot_idx = self.tot_blocks * P
        self.ch_max = ch_max

        # order edges: key = (core, w, half); stable sort
        key = (core * W + w) * 2 + half
        order = np.argsort(key, kind="stable")
        s_core, s_w, s_half = core[order], w[order], half[order]
        s_lsrc, s_lidx = lsrc[order], lidx[order]

        # block offsets per (w, half) in the packed stream (same per core)
        blk_off = np.zeros((W, 2), dtype=np.int64)
        acc = 0
        for wi in range(W):
            for hi in range(2):
                blk_off[wi, hi] = acc
                acc += self.cap_blocks[wi, hi]
        self.blk_off = blk_off

        # rank within each (core,w,half) group: groups are contiguous after sort
        gkey = (s_core * W + s_w) * 2 + s_half
        change = np.r_[True, gkey[1:] != gkey[:-1]]
        grp_start = np.flatnonzero(change)
        grp_id = np.cumsum(change) - 1
        grp_rank = np.arange(len(order)) - grp_start[grp_id]
        slot = blk_off[s_w, s_half] * P + grp_rank  # global slot within core stream

        # call table: chunks of <=ch_max blocks per (w, half). Order: the
        # first `warm_windows` windows issue all their h0 chunks before any
        # h1 chunk (h0 gathers overlap the y1-build tail of phase 1); the
        # rest interleave [h0 chunks, h1 chunks] per window.
        def chunks_for(wi, hi):
            c = int(self.cap_blocks[wi, hi])
            b0 = int(blk_off[wi, hi])
            out = []
            off = 0
            while off < c:
                nb = min(ch_max, c - off)
                out.append((wi, hi, b0 + off, nb))
                off += nb
            return out

        WARM = min(warm_windows, W)
        calls = []
        for wi in range(WARM):
            calls += chunks_for(wi, 0)
        for wi in range(WARM):
            calls += chunks_for(wi, 1)
        for wi in range(WARM, W):
            calls += chunks_for(wi, 0) + chunks_for(wi, 1)
        self.calls = calls
        self.nblk_w = self.cap_blocks.sum(axis=1)  # blocks per window

        # pack idx into wrapped [16, tot_idx/16] (call-granular): within a call
        # starting at slot g0 (mult of 128), element i -> [i%16, g0//16 + i//16]
        self.idx_packed = np.zeros((n_cores, 128, self.tot_idx // 16), np.int16)
        self.meta_packed = np.full((n_cores, P, self.tot_blocks), -1.0, np.float32)
        call_starts = np.array(sorted(b0 * P for (_, _, b0, nb) in calls),
                               dtype=np.int64)
        call_of_slot_idx = np.searchsorted(call_starts, slot, side="right") - 1
        g0 = call_starts[call_of_slot_idx]
        i_in_call = slot - g0
        row16 = i_in_call % 16
        col16 = g0 // 16 + i_in_call // 16
        self.idx_packed[s_core, row16, col16] = s_lidx.astype(np.int16)
        # HW: each of the 8 GpSimd cores reads indices from its own
        # 16-partition group -> replicate the 16-row pattern across all 128.
        self.idx_packed[:, 16:, :] = np.tile(self.idx_packed[:, :16, :], (1, 7, 1))
        blk = slot // P
        pslot = slot % P
        self.meta_packed[s_core, pslot, blk] = s_lsrc.astype(np.float32)

        self.max_group = int(cap.max())
        self.pad_frac = (self.tot_idx * n_cores) / max(1, len(src)) - 1.0


def build_program(cfg: Config):
    nc = bacc.Bacc("TRN2", target_bir_lowering=False, debug=False,
                   num_devices=cfg.n_cores, num_swdge_queues=4)
    DTY = cfg.dt_y
    W = cfg.w_per_core

    # x^T, cast to bf16 on host: [IN_FEAT, n_pad]
    xt_d = nc.dram_tensor("xt", [IN_FEAT, cfg.n_pad], BF16, kind="ExternalInput")
    wcat_d = nc.dram_tensor("wcat", [IN_FEAT, TD], BF16, kind="ExternalInput")
    war_d = nc.dram_tensor("war", [IN_FEAT, HEADS], BF16, kind="ExternalInput")
    iota_d = nc.dram_tensor("iota", [P, P], cfg.dt_meta, kind="ExternalInput")
    idx_d = nc.dram_tensor("idx", [128, cfg.tot_idx // 16], I16, kind="ExternalInput")
    meta_d = nc.dram_tensor("meta", [P, cfg.tot_blocks], cfg.dt_meta,
                            kind="ExternalInput")
    out_d = nc.dram_tensor("out", [cfg.npc, TD], F32, kind="ExternalOutput")
    y0_d = nc.dram_tensor("y0", [cfg.h0_rows, YW], DTY, kind="Internal")
    y1_d = nc.dram_tensor("y1", [cfg.h1_rows, YW], DTY, kind="Internal")

    y_writes = [[], []]  # per half
    with TileContext(nc) as tc:
        with ExitStack() as octx:
            # phase-2 constants load first so the warm h0 gathers are not
            # stuck behind phase-1 DMAs on the Sync queue
            consts2 = octx.enter_context(tc.tile_pool(name="consts2", bufs=1))
            iota = consts2.tile([P, P], cfg.dt_meta)
            nc.sync.dma_start(iota[:, :], iota_d[:, :])
            idx_sb = consts2.tile([128, cfg.tot_idx // 16], I16, tag="idx")
            nc.sync.dma_start(idx_sb[:, :], idx_d[:, :])
            meta_sb = consts2.tile([P, cfg.tot_blocks], cfg.dt_meta, tag="meta")
            nc.sync.dma_start(meta_sb[:, :], meta_d[:, :])
            neg1 = consts2.tile([P, 1], F32, tag="neg1")
            nc.vector.memset(neg1[:, :], -1.0)

            # ---------------- phase 1: build Y table ----------------
            with ExitStack() as ctx:
                consts = ctx.enter_context(tc.tile_pool(name="consts", bufs=1))
                wc = consts.tile([P, 2, TD], BF16, tag="wc")
                nc.sync.dma_start(wc[:, :, :],
                                  wcat_d.rearrange("(c p) n -> p c n", p=P))
                wr = consts.tile([P, 2, HEADS], BF16, tag="wr")
                nc.sync.dma_start(wr[:, :, :],
                                  war_d.rearrange("(c p) n -> p c n", p=P))

                xin = ctx.enter_context(tc.tile_pool(name="xin", bufs=4))
                yout = ctx.enter_context(tc.tile_pool(name="yout", bufs=4))
                ps_t = ctx.enter_context(
                    tc.tile_pool(name="ps_t", bufs=3, space="PSUM"))
                ps_p = ctx.enter_context(
                    tc.tile_pool(name="ps_p", bufs=3, space="PSUM"))

                xt_v = xt_d.rearrange("(c p) n -> p c n", p=P)
                groups = []
                for hsel, (t0, ntiles) in enumerate(
                        [(0, cfg.h0_tiles), (cfg.h0_tiles,
                                             cfg.x_tiles - cfg.h0_tiles)]):
                    off = 0
                    while off < ntiles:
                        tb = min(TB, ntiles - off)
                        groups.append((hsel, t0 + off, off, tb))
                        off += tb
                for (hsel, tg, tloc, tb) in groups:
                    xT = xin.tile([P, 2, TB, P], BF16)
                    nc.sync.dma_start(
                        xT[:, :, 0:tb, :],
                        xt_v[:, :, tg * P:(tg + tb) * P].rearrange(
                            "p c (s q) -> p c s q", s=tb))
                    ysb = yout.tile([P, TB, YW], DTY)
                    for s in range(tb):
                        pt = ps_t.tile([P, 512], F32, tag="pt")
                        par = ps_p.tile([P, 512], F32, tag="par")
                        nc.tensor.matmul(par[:, 0:HEADS], xT[:, 0, s, :],
                                         wr[:, 0, :], start=True, stop=False)
                        nc.tensor.matmul(par[:, 0:HEADS], xT[:, 1, s, :],
                                         wr[:, 1, :], start=False, stop=True)
                        nc.tensor.matmul(pt[:, :], xT[:, 0, s, :], wc[:, 0, :],
                                         start=True, stop=False)
                        nc.tensor.matmul(pt[:, :], xT[:, 1, s, :], wc[:, 1, :],
                                         start=False, stop=True)
                        # Row layout: [t~ h0-3 (256) | u h0-3 (4) | 12 pad |
                        # t~ h4-7 (256) | u h4-7 (4) | pad]; group stride
                        # GS=272. wcat is permuted on host so pt columns are
                        # [t~ h0-3 | t~ h4-7] contiguous.
                        yv = ysb[:, s, 0:YS].rearrange("p (g c) -> p g c", g=2)
                        # u = exp(Ar) into the two 4-col slices
                        nc.scalar.activation(
                            yv[:, :, 256:260],
                            par[:, 0:HEADS].rearrange("p (g c) -> p g c", g=2),
                            mybir.ActivationFunctionType.Exp)
                        # t~ = t * u (broadcast u over the 64 dims of each head)
                        nc.vector.tensor_tensor(
                            yv[:, :, 0:256].rearrange("p g (h o) -> p g h o", h=4),
                            pt[:, :].rearrange("p (g h o) -> p g h o", g=2, h=4),
                            yv[:, :, 256:260].unsqueeze(3).broadcast_to(
                                [P, 2, 4, OUT]),
                            mybir.AluOpType.mult,
                        )
                    # write tb full 1280B rows in one DMA: the destination
                    # region is fully contiguous (best HWDGE descriptor
                    # shape); pad cols carry garbage that is never read.
                    # ScalarE HWDGE so SyncE keeps issuing x loads.
                    y_d = y0_d if hsel == 0 else y1_d
                    dst = y_d[tloc * P:(tloc + tb) * P, :]
                    wi_ = nc.scalar.dma_start(
                        dst.rearrange("(s r) c -> r s c", s=tb),
                        ysb[:, 0:tb, :])
                    y_writes[hsel].append(wi_)

            # ---------------- phase 2: gather + segment sums ----------------
            with ExitStack() as ctx:
                gpool = ctx.enter_context(tc.tile_pool(name="gath", bufs=10))
                spool = ctx.enter_context(tc.tile_pool(name="onehot", bufs=4))
                opool = ctx.enter_context(tc.tile_pool(name="outp", bufs=2))
                ps_num = ctx.enter_context(
                    tc.tile_pool(name="ps_num", bufs=4, space="PSUM"))
                ps_den = ctx.enter_context(
                    tc.tile_pool(name="ps_den", bufs=4, space="PSUM"))

                fence_pending = [True, True]  # per half
                win_state = {}  # wi -> [pn0, pn1, bi]

                for ci, (wi, hi, b0, nb) in enumerate(cfg.calls):
                    if wi not in win_state:
                        pn0 = ps_num.tile([P, 512], F32, tag="pn0")
                        pn1 = ps_den.tile([P, 512], F32, tag="pn1")
                        win_state[wi] = [pn0, pn1, 0]
                    pn0, pn1, bi = win_state[wi]
                    nblk_w = int(cfg.nblk_w[wi])

                    g = gpool.tile([P, cfg.ch_max, YS], DTY)
                    src_t = y0_d if hi == 0 else y1_d
                    g_inst = dma_gather_span(
                        nc.gpsimd,
                        out_ap=g[:, 0:nb, :],
                        in_ap=src_t[:, 0:YS],
                        idxs_ap=idx_sb[:, b0 * 8:(b0 + nb) * 8],
                        num_idxs=nb * P,
                        num_idxs_reg=nb * P,
                        elem_size=YS,
                        elem_step=YW,
                        queue_num=ci % 4,
                    )
                    if fence_pending[hi]:
                        # phase fence: the gather's indexed DRAM read of the Y
                        # tables is invisible to Tile's dependency tracking;
                        # gathers run in order on GpSimd, so gating the first
                        # gather per half on that half's writes fences it.
                        for wr_ in y_writes[hi]:
                            add_dep_helper(g_inst.ins, wr_.ins,
                                           reason="gather reads Y table")
                        fence_pending[hi] = False
                    s = spool.tile([P, cfg.ch_max, P], cfg.dt_s)
                    nc.vector.tensor_tensor(
                        s[:, 0:nb, :],
                        meta_sb[:, b0:b0 + nb].unsqueeze(2).broadcast_to(
                            [P, nb, P]),
                        iota[:, :].unsqueeze(1).broadcast_to([P, nb, P]),
                        mybir.AluOpType.is_equal,
                    )
                    for j in range(nb):
                        st = (bi == 0)
                        sp = (bi == nblk_w - 1)
                        nc.tensor.matmul(pn0[:, 0:GW], s[:, j, :],
                                         g[:, j, 0:GW],
                                         start=st, stop=sp,
                                         skip_group_check=True)
                        nc.tensor.matmul(pn1[:, 0:GW], s[:, j, :],
                                         g[:, j, GS:GS + GW],
                                         start=st, stop=sp,
                                         skip_group_check=True)
                        bi += 1
                    win_state[wi][2] = bi

                    if bi < nblk_w:
                        continue
                    # ---- evict window ----
                    del win_state[wi]
                    den = opool.tile([P, 2, 4], F32, tag="den")
                    nc.vector.tensor_scalar_add(den[:, 0, :], pn0[:, 256:260],
                                                1e-30)
                    nc.vector.tensor_scalar_add(den[:, 1, :], pn1[:, 256:260],
                                                1e-30)
                    rden = opool.tile([P, 2, 4], F32, tag="rden")
                    nc.vector.reciprocal(rden[:, :, :], den[:, :, :])
                    hout = opool.tile([P, TD], F32, tag="hout")
                    hv = hout[:, :].rearrange("p (g h o) -> p g h o", g=2, h=4)
                    nc.vector.tensor_tensor(
                        hv[:, 0, :, :],
                        pn0[:, 0:256].rearrange("p (h o) -> p h o", h=4),
                        rden[:, 0, :].unsqueeze(2).broadcast_to([P, 4, OUT]),
                        mybir.AluOpType.mult,
                    )
                    nc.vector.tensor_tensor(
                        hv[:, 1, :, :],
                        pn1[:, 0:256].rearrange("p (h o) -> p h o", h=4),
                        rden[:, 1, :].unsqueeze(2).broadcast_to([P, 4, OUT]),
                        mybir.AluOpType.mult,
                    )
                    # elu(z) = max(z,0) + exp(min(z,0)) - 1
                    # min(z,0) = -relu(-z); both steps on ScalarE to dodge the
                    # DVE<->GpSimd shared-SBUF-port lock during gather desc-gen.
                    xm = opool.tile([P, TD], F32, tag="xm")
                    nc.scalar.activation(xm[:, :], hout[:, :],
                                         mybir.ActivationFunctionType.Relu,
                                         scale=-1.0)
                    ex = opool.tile([P, TD], F32, tag="ex")
                    nc.scalar.activation(ex[:, :], xm[:, :],
                                         mybir.ActivationFunctionType.Exp,
                                         scale=-1.0)
                    fin = opool.tile([P, TD], F32, tag="fin")
                    nc.vector.scalar_tensor_tensor(
                        out=fin[:, :], in0=hout[:, :], scalar=0.0, in1=ex[:, :],
                        op0=mybir.AluOpType.max, op1=mybir.AluOpType.add,
                    )
                    fin2 = opool.tile([P, TD], F32, tag="fin2")
                    nc.scalar.activation(fin2[:, :], fin[:, :],
                                         mybir.ActivationFunctionType.Identity,
                                         bias=neg1[:, :])
                    nc.sync.dma_start(out_d[wi * P:(wi + 1) * P, :], fin2[:, :])

    nc.compile()
    return nc


def host_prep(cfg: Config, x, Ws, As):
    import ml_dtypes as _md
    x = np.asarray(x, np.float32)
    Ws = np.asarray(Ws, np.float32)
    As = np.asarray(As, np.float32)
    n = x.shape[0]
    xt = np.zeros((IN_FEAT, cfg.n_pad), np.float32)
    xt[:, :n] = x.T
    xt = xt.astype(_md.bfloat16)
    # wcat[f, h*64+o] = Ws[h,o,f]
    wcat = Ws.transpose(2, 0, 1).reshape(IN_FEAT, TD).astype(_md.bfloat16)
    a_r = As[:, OUT:, 0]  # [H, O]
    war = np.einsum("hof,ho->fh", Ws, a_r).astype(_md.bfloat16)
    iota = np.tile(np.arange(P, dtype=np.float32), (P, 1))
    iota = iota.astype(_md.bfloat16)
    meta = cfg.meta_packed.astype(_md.bfloat16)
    in_maps = []
    for c in range(cfg.n_cores):
        in_maps.append({
            "xt": xt, "wcat": wcat, "war": war,
            "iota": np.ascontiguousarray(iota),
            "idx": np.ascontiguousarray(cfg.idx_packed[c]),
            "meta": np.ascontiguousarray(meta[c]),
        })
    return in_maps


from concourse.bass_utils import run_bass_kernel_spmd

LAST_EXEC_TIME_NS = None


def kernel(x, src, dst, Ws, As):
    """Full-input entry point: shards internally across 8 NeuronCores."""
    global LAST_EXEC_TIME_NS
    x = np.asarray(x, np.float32)
    src = np.asarray(src)
    dst = np.asarray(dst)
    Ws = np.asarray(Ws, np.float32)
    As = np.asarray(As, np.float32)
    n = x.shape[0]

    cfg = Config(n, src, dst, n_cores=8)
    nc = build_program(cfg)
    in_maps = host_prep(cfg, x, Ws, As)
    import os as _os
    _trace = _os.environ.get("KERNEL_TRACE", "0") == "1"
    _tdir = _os.environ.get("KERNEL_TRACE_DIR") or None
    res = run_bass_kernel_spmd(nc, in_maps, core_ids=list(range(cfg.n_cores)),
                               trace=_trace, tmpdir=_tdir)
    LAST_EXEC_TIME_NS = res.exec_time_ns
    out = np.concatenate([res.results[c]["out"] for c in range(cfg.n_cores)],
                         axis=0)[:n]
    return np.ascontiguousarray(out, dtype=np.float32)


# revision 35
# speedup vs baseline: 1.2486x; 1.2486x over previous
"""GAT message-passing kernel for TRN2 (8-core SPMD).

Math (heads h, nodes n):
  t[n,h,:] = x[n] @ Ws[h].T            (t-space features, 64 per head)
  Ar[n,h]  = x[n] @ war[:,h]           (war = Ws[h].T @ a_r[h], folded weights)
  u        = exp(Ar)
  out[i, h*64:h*64+64] = elu( sum_{e:src=i} u[dst,h]*t[dst,h,:] / sum u[dst,h] )

Sharding: src-range per core. Phase 1 builds the Y table
[t~ h0-3 (256) | u h0-3 (4) | pad | t~ h4-7 | u h4-7 | pad] for all nodes
(replicated on every core, in core-private DRAM, split in two halves so
gather indices fit int16). Phase 2 gathers per-edge Y rows (edges sorted by
(window, half), padded to 128-edge blocks), builds a one-hot S on DVE, and
segment-sums via PE matmul into PSUM per 128-node window; then
normalize + elu + store.

Perf notes (v3):
- Phase 1 batches 8 node-tiles per DMA: x-loads on SyncE (2KB runs), full
  1280B-row Y-writes on ScalarE (fully contiguous destination region) so
  HWDGE issue time and descriptor overhead stay off the critical path.
- Gathers fetch only the used 544-col span of each 640-col row
  (elem 1088B, stride 1280B): the bass-level elem%256 assert is bypassed
  via direct InstDMAGatherAnt construction — the Q7 kernel computes byte
  lengths directly; only the row *stride* is encoded in 256B units.
- Phase 2 splits each (window, half) gather into <=9-block calls and
  round-robins the 4 SWDGE queues (concurrent desc-gen on 4 Q7 core
  pairs). Deep pools (gather bufs=10, psum 4+4 full banks) keep 4 windows
  in flight.
- idx/meta/iota load before phase 1; the first 4 windows issue their
  half-0 gathers before any half-1 gather so y0-table gathers overlap the
  y1-build tail of phase 1 (a half-1 call earlier would head-of-line-block
  the in-order Pool queue on the y1 fence).
"""

import math
import numpy as np
from contextlib import ExitStack

import concourse.bass as bass
import concourse.bacc as bacc
import concourse.mybir as mybir
from concourse.tile import TileContext
from concourse.tile import add_dep_helper

F32 = mybir.dt.float32
BF16 = mybir.dt.bfloat16
FP8 = mybir.dt.float8e4
I16 = mybir.dt.int16

P = 128
IN_FEAT = 256
HEADS = 8
OUT = 64
TD = HEADS * OUT  # 512
GW = 260   # matmul N per group: 4 heads x 64 + 4 u columns
GS = 272   # group stride in the Y row (544B, 32B-aligned for PE rhs)
YS = 2 * GS  # gathered span per row: 544 cols = 1088B
YW = 640   # DRAM row stride: 1280B (multiple of 256B, required by gather)
TB = 8     # phase-1 node-tiles per DMA batch


def dma_gather_span(eng, out_ap, in_ap, idxs_ap, num_idxs, num_idxs_reg,
                    elem_size, elem_step, queue_num):
    """nc.gpsimd.dma_gather minus the elem_size%256 restriction.

    The Q7 dma_gather kernel computes descriptor byte lengths as
    elem_size*dtype_size directly (any value works); only the row *stride*
    is ISA-encoded in 256B units. bass's helper asserts elem%256 for the
    transpose path and applies it unconditionally, so build the
    instruction with the same lowering calls the helper makes.
    """
    eng._assert_queue_num(queue_num)
    assert idxs_ap.dtype == mybir.dt.int16
    assert in_ap.dtype == out_ap.dtype
    dt_size = mybir.dt.size(in_ap.dtype)
    stride_bytes = elem_step * dt_size
    assert stride_bytes % 256 == 0 and stride_bytes // 256 < 256
    assert (elem_size * dt_size) % 32 == 0
    assert in_ap.ap[0][0] == elem_step
    assert in_ap.ap[-1][1] == out_ap.ap[-1][1] == elem_size
    _in_ap = eng.lower_ap_dma(in_ap, for_custom_bir_dma=True)
    _idxs_ap = eng.lower_ap(idxs_ap)
    _out_ap = eng.lower_ap(out_ap)
    return eng.add_instruction(
        mybir.InstDMAGatherAnt(
            name=eng.bass.get_next_instruction_name(),
            ins=[*_in_ap, _idxs_ap,
                 eng.lower_val_access(eng.to_reg(num_idxs_reg))],
            outs=[_out_ap],
            transpose=False,
            num_idxs=num_idxs,
            elem_size=elem_size,
            stride_bytes_256=stride_bytes // 256,
            gen_mode=0,
            single_packet=False,
            queue_num=queue_num,
            sbuf_tokens_per_rank=0,
            sbuf_free_dim_per_rank=0,
            sbuf_free_dim_pad_per_rank=0,
            sbuf_byte_offset=0,
        )
    )


class Config:
    def __init__(self, n_nodes, src, dst, n_cores=8, ch_max=9, warm_windows=4):
        self.n_cores = n_cores
        self.dt_y = BF16
        self.dt_meta = BF16
        self.dt_s = FP8

        self.n_nodes = n_nodes
        # nodes per core, multiple of 128
        self.w_per_core = math.ceil(n_nodes / (n_cores * P))
        self.npc = self.w_per_core * P
        self.n_pad = self.npc * n_cores          # padded node count (Y rows)
        self.x_tiles = self.n_pad // P
        # half split for int16 gather indices
        h0_tiles = self.x_tiles // 2
        assert h0_tiles * P < 32768 and (self.x_tiles - h0_tiles) * P < 32768
        self.h0_tiles = h0_tiles
        self.h0_rows = h0_tiles * P
        self.h1_rows = (self.x_tiles - h0_tiles) * P

        # ---- per-core edge grouping (by window, dst-half) ----
        W = self.w_per_core
        src = np.asarray(src, dtype=np.int64)
        dst = np.asarray(dst, dtype=np.int64)
        core = src // self.npc
        w = (src % self.npc) // P
        lsrc = src % P
        half = (dst >= self.h0_rows).astype(np.int64)
        lidx = dst - self.h0_rows * half

        counts = np.zeros((n_cores, W, 2), dtype=np.int64)
        np.add.at(counts, (core, w, half), 1)
        cap = counts.max(axis=0)  # [W, 2] max over cores
        self.cap_blocks = np.ceil(cap / P).astype(np.int64)  # C[w][h]
        self.tot_blocks = int(self.cap_blocks.sum())
        self.tot_idx = self.tot_blocks * P
        self.ch_max = ch_max

        # order edges: key = (core, w, half), secondary sort by dst row so
        # each gather call reads monotonically increasing DRAM addresses
        key = (core * W + w) * 2 + half
        order = np.lexsort((lidx, key))
        s_core, s_w, s_half = core[order], w[order], half[order]
        s_lsrc, s_lidx = lsrc[order], lidx[order]

        # block offsets per (w, half) in the packed stream (same per core)
        blk_off = np.zeros((W, 2), dtype=np.int64)
        acc = 0
        for wi in range(W):
            for hi in range(2):
                blk_off[wi, hi] = acc
                acc += self.cap_blocks[wi, hi]
        self.blk_off = blk_off

        # rank within each (core,w,half) group: groups are contiguous after sort
        gkey = (s_core * W + s_w) * 2 + s_half
        change = np.r_[True, gkey[1:] != gkey[:-1]]
        grp_start = np.flatnonzero(change)
        grp_id = np.cumsum(change) - 1
        grp_rank = np.arange(len(order)) - grp_start[grp_id]
        slot = blk_off[s_w, s_half] * P + grp_rank  # global slot within core stream

        # call table: chunks of <=ch_max blocks per (w, half). Order: the
        # first `warm_windows` windows issue all their h0 chunks before any
        # h1 chunk (h0 gathers overlap the y1-build tail of phase 1); the
        # rest interleave [h0 chunks, h1 chunks] per window.
        def chunks_for(wi, hi):
            c = int(self.cap_blocks[wi, hi])
            b0 = int(blk_off[wi, hi])
            out = []
            off = 0
            while off < c:
                nb = min(ch_max, c - off)
                out.append((wi, hi, b0 + off, nb))
                off += nb
            return out

        WARM = min(warm_windows, W)
        calls = []
        for wi in range(WARM):
            calls += chunks_for(wi, 0)
        for wi in range(WARM):
            calls += chunks_for(wi, 1)
        for wi in range(WARM, W):
            calls += chunks_for(wi, 0) + chunks_for(wi, 1)
        self.calls = calls
        self.nblk_w = self.cap_blocks.sum(axis=1)  # blocks per window

        # pack idx into wrapped [16, tot_idx/16] (call-granular): within a call
        # starting at slot g0 (mult of 128), element i -> [i%16, g0//16 + i//16]
        self.idx_packed = np.zeros((n_cores, 128, self.tot_idx // 16), np.int16)
        self.meta_packed = np.full((n_cores, P, self.tot_blocks), -1.0, np.float32)
        call_starts = np.array(sorted(b0 * P for (_, _, b0, nb) in calls),
                               dtype=np.int64)
        call_of_slot_idx = np.searchsorted(call_starts, slot, side="right") - 1
        g0 = call_starts[call_of_slot_idx]
        i_in_call = slot - g0
        row16 = i_in_call % 16
        col16 = g0 // 16 + i_in_call // 16
        self.idx_packed[s_core, row16, col16] = s_lidx.astype(np.int16)
        # HW: each of the 8 GpSimd cores reads indices from its own
        # 16-partition group -> replicate the 16-row pattern across all 128.
        self.idx_packed[:, 16:, :] = np.tile(self.idx_packed[:, :16, :], (1, 7, 1))
        blk = slot // P
        pslot = slot % P
        self.meta_packed[s_core, pslot, blk] = s_lsrc.astype(np.float32)

        self.max_group = int(cap.max())
        self.pad_frac = (self.tot_idx * n_cores) / max(1, len(src)) - 1.0


def build_program(cfg: Config):
    nc = bacc.Bacc("TRN2", target_bir_lowering=False, debug=False,
                   num_devices=cfg.n_cores, num_swdge_queues=4)
    DTY = cfg.dt_y
    W = cfg.w_per_core

    # x^T, cast to bf16 on host: [IN_FEAT, n_pad]
    xt_d = nc.dram_tensor("xt", [IN_FEAT, cfg.n_pad], BF16, kind="ExternalInput")
    wcat_d = nc.dram_tensor("wcat", [IN_FEAT, TD], BF16, kind="ExternalInput")
    war_d = nc.dram_tensor("war", [IN_FEAT, HEADS], BF16, kind="ExternalInput")
    iota_d = nc.dram_tensor("iota", [P, P], cfg.dt_meta, kind="ExternalInput")
    idx_d = nc.dram_tensor("idx", [128, cfg.tot_idx // 16], I16, kind="ExternalInput")
    meta_d = nc.dram_tensor("meta", [P, cfg.tot_blocks], cfg.dt_meta,
                            kind="ExternalInput")
    # bf16 output (host upcasts): halves the 12.8MB/core of result traffic;
    # the ~0.2% rounding is well inside the 2e-2 tolerance
    out_d = nc.dram_tensor("out", [cfg.npc, TD], BF16, kind="ExternalOutput")
    y0_d = nc.dram_tensor("y0", [cfg.h0_rows, YW], DTY, kind="Internal")
    y1_d = nc.dram_tensor("y1", [cfg.h1_rows, YW], DTY, kind="Internal")

    y_writes = [[], []]  # per half
    with TileContext(nc) as tc:
        with ExitStack() as octx:
            # phase-2 constants load first so the warm h0 gathers are not
            # stuck behind phase-1 DMAs on the Sync queue
            consts2 = octx.enter_context(tc.tile_pool(name="consts2", bufs=1))
            iota = consts2.tile([P, P], cfg.dt_meta)
            nc.sync.dma_start(iota[:, :], iota_d[:, :])
            idx_sb = consts2.tile([128, cfg.tot_idx // 16], I16, tag="idx")
            nc.sync.dma_start(idx_sb[:, :], idx_d[:, :])
            meta_sb = consts2.tile([P, cfg.tot_blocks], cfg.dt_meta, tag="meta")
            nc.sync.dma_start(meta_sb[:, :], meta_d[:, :])
            neg1 = consts2.tile([P, 1], F32, tag="neg1")
            nc.vector.memset(neg1[:, :], -1.0)

            # ---------------- phase 1: build Y table ----------------
            with ExitStack() as ctx:
                consts = ctx.enter_context(tc.tile_pool(name="consts", bufs=1))
                wc = consts.tile([P, 2, TD], BF16, tag="wc")
                nc.sync.dma_start(wc[:, :, :],
                                  wcat_d.rearrange("(c p) n -> p c n", p=P))
                wr = consts.tile([P, 2, HEADS], BF16, tag="wr")
                nc.sync.dma_start(wr[:, :, :],
                                  war_d.rearrange("(c p) n -> p c n", p=P))

                xin = ctx.enter_context(tc.tile_pool(name="xin", bufs=4))
                yout = ctx.enter_context(tc.tile_pool(name="yout", bufs=4))
                ps_t = ctx.enter_context(
                    tc.tile_pool(name="ps_t", bufs=3, space="PSUM"))
                ps_p = ctx.enter_context(
                    tc.tile_pool(name="ps_p", bufs=3, space="PSUM"))

                xt_v = xt_d.rearrange("(c p) n -> p c n", p=P)
                groups = []
                for hsel, (t0, ntiles) in enumerate(
                        [(0, cfg.h0_tiles), (cfg.h0_tiles,
                                             cfg.x_tiles - cfg.h0_tiles)]):
                    off = 0
                    while off < ntiles:
                        tb = min(TB, ntiles - off)
                        groups.append((hsel, t0 + off, off, tb))
                        off += tb
                for (hsel, tg, tloc, tb) in groups:
                    xT = xin.tile([P, 2, TB, P], BF16)
                    nc.sync.dma_start(
                        xT[:, :, 0:tb, :],
                        xt_v[:, :, tg * P:(tg + tb) * P].rearrange(
                            "p c (s q) -> p c s q", s=tb))
                    ysb = yout.tile([P, TB, YS], DTY)
                    for s in range(tb):
                        pt = ps_t.tile([P, 512], F32, tag="pt")
                        par = ps_p.tile([P, 512], F32, tag="par")
                        nc.tensor.matmul(par[:, 0:HEADS], xT[:, 0, s, :],
                                         wr[:, 0, :], start=True, stop=False)
                        nc.tensor.matmul(par[:, 0:HEADS], xT[:, 1, s, :],
                                         wr[:, 1, :], start=False, stop=True)
                        nc.tensor.matmul(pt[:, :], xT[:, 0, s, :], wc[:, 0, :],
                                         start=True, stop=False)
                        nc.tensor.matmul(pt[:, :], xT[:, 1, s, :], wc[:, 1, :],
                                         start=False, stop=True)
                        # Row layout: [t~ h0-3 (256) | u h0-3 (4) | 12 pad |
                        # t~ h4-7 (256) | u h4-7 (4) | pad]; group stride
                        # GS=272. wcat is permuted on host so pt columns are
                        # [t~ h0-3 | t~ h4-7] contiguous.
                        yv = ysb[:, s, 0:YS].rearrange("p (g c) -> p g c", g=2)
                        # u = exp(Ar) into the two 4-col slices
                        nc.scalar.activation(
                            yv[:, :, 256:260],
                            par[:, 0:HEADS].rearrange("p (g c) -> p g c", g=2),
                            mybir.ActivationFunctionType.Exp)
                        # t~ = t * u (broadcast u over the 64 dims of each head)
                        nc.vector.tensor_tensor(
                            yv[:, :, 0:256].rearrange("p g (h o) -> p g h o", h=4),
                            pt[:, :].rearrange("p (g h o) -> p g h o", g=2, h=4),
                            yv[:, :, 256:260].unsqueeze(3).broadcast_to(
                                [P, 2, 4, OUT]),
                            mybir.AluOpType.mult,
                        )
                    # write tb rows' used 1088B span in one DMA. ScalarE
                    # HWDGE so SyncE keeps issuing x loads.
                    y_d = y0_d if hsel == 0 else y1_d
                    dst = y_d[tloc * P:(tloc + tb) * P, 0:YS]
                    wi_ = nc.scalar.dma_start(
                        dst.rearrange("(s r) c -> r s c", s=tb),
                        ysb[:, 0:tb, :])
                    y_writes[hsel].append(wi_)

            # ---------------- phase 2: gather + segment sums ----------------
            with ExitStack() as ctx:
                gpool = ctx.enter_context(tc.tile_pool(name="gath", bufs=10))
                spool = ctx.enter_context(tc.tile_pool(name="onehot", bufs=4))
                opool = ctx.enter_context(tc.tile_pool(name="outp", bufs=2))
                ps_num = ctx.enter_context(
                    tc.tile_pool(name="ps_num", bufs=4, space="PSUM"))
                ps_den = ctx.enter_context(
                    tc.tile_pool(name="ps_den", bufs=4, space="PSUM"))

                fence_pending = [True, True]  # per half
                win_state = {}  # wi -> [pn0, pn1, bi]

                for ci, (wi, hi, b0, nb) in enumerate(cfg.calls):
                    if wi not in win_state:
                        pn0 = ps_num.tile([P, 512], F32, tag="pn0")
                        pn1 = ps_den.tile([P, 512], F32, tag="pn1")
                        win_state[wi] = [pn0, pn1, 0]
                    pn0, pn1, bi = win_state[wi]
                    nblk_w = int(cfg.nblk_w[wi])

                    g = gpool.tile([P, cfg.ch_max, YS], DTY)
                    src_t = y0_d if hi == 0 else y1_d
                    g_inst = dma_gather_span(
                        nc.gpsimd,
                        out_ap=g[:, 0:nb, :],
                        in_ap=src_t[:, 0:YS],
                        idxs_ap=idx_sb[:, b0 * 8:(b0 + nb) * 8],
                        num_idxs=nb * P,
                        num_idxs_reg=nb * P,
                        elem_size=YS,
                        elem_step=YW,
                        queue_num=ci % 4,
                    )
                    if fence_pending[hi]:
                        # phase fence: the gather's indexed DRAM read of the Y
                        # tables is invisible to Tile's dependency tracking;
                        # gathers run in order on GpSimd, so gating the first
                        # gather per half on that half's writes fences it.
                        for wr_ in y_writes[hi]:
                            add_dep_helper(g_inst.ins, wr_.ins,
                                           reason="gather reads Y table")
                        fence_pending[hi] = False
                    s = spool.tile([P, cfg.ch_max, P], cfg.dt_s)
                    nc.vector.tensor_tensor(
                        s[:, 0:nb, :],
                        meta_sb[:, b0:b0 + nb].unsqueeze(2).broadcast_to(
                            [P, nb, P]),
                        iota[:, :].unsqueeze(1).broadcast_to([P, nb, P]),
                        mybir.AluOpType.is_equal,
                    )
                    for j in range(nb):
                        st = (bi == 0)
                        sp = (bi == nblk_w - 1)
                        nc.tensor.matmul(pn0[:, 0:GW], s[:, j, :],
                                         g[:, j, 0:GW],
                                         start=st, stop=sp,
                                         skip_group_check=True)
                        nc.tensor.matmul(pn1[:, 0:GW], s[:, j, :],
                                         g[:, j, GS:GS + GW],
                                         start=st, stop=sp,
                                         skip_group_check=True)
                        bi += 1
                    win_state[wi][2] = bi

                    if bi < nblk_w:
                        continue
                    # ---- evict window ----
                    del win_state[wi]
                    den = opool.tile([P, 2, 4], F32, tag="den")
                    nc.vector.tensor_scalar_add(den[:, 0, :], pn0[:, 256:260],
                                                1e-30)
                    nc.vector.tensor_scalar_add(den[:, 1, :], pn1[:, 256:260],
                                                1e-30)
                    rden = opool.tile([P, 2, 4], F32, tag="rden")
                    nc.vector.reciprocal(rden[:, :, :], den[:, :, :])
                    hout = opool.tile([P, TD], F32, tag="hout")
                    hv = hout[:, :].rearrange("p (g h o) -> p g h o", g=2, h=4)
                    nc.vector.tensor_tensor(
                        hv[:, 0, :, :],
                        pn0[:, 0:256].rearrange("p (h o) -> p h o", h=4),
                        rden[:, 0, :].unsqueeze(2).broadcast_to([P, 4, OUT]),
                        mybir.AluOpType.mult,
                    )
                    nc.vector.tensor_tensor(
                        hv[:, 1, :, :],
                        pn1[:, 0:256].rearrange("p (h o) -> p h o", h=4),
                        rden[:, 1, :].unsqueeze(2).broadcast_to([P, 4, OUT]),
                        mybir.AluOpType.mult,
                    )
                    # elu(z) = max(z,0) + exp(min(z,0)) - 1
                    # min(z,0) = -relu(-z); both steps on ScalarE to dodge the
                    # DVE<->GpSimd shared-SBUF-port lock during gather desc-gen.
                    xm = opool.tile([P, TD], F32, tag="xm")
                    nc.scalar.activation(xm[:, :], hout[:, :],
                                         mybir.ActivationFunctionType.Relu,
                                         scale=-1.0)
                    ex = opool.tile([P, TD], F32, tag="ex")
                    nc.scalar.activation(ex[:, :], xm[:, :],
                                         mybir.ActivationFunctionType.Exp,
                                         scale=-1.0)
                    fin = opool.tile([P, TD], F32, tag="fin")
                    nc.vector.scalar_tensor_tensor(
                        out=fin[:, :], in0=hout[:, :], scalar=0.0, in1=ex[:, :],
                        op0=mybir.AluOpType.max, op1=mybir.AluOpType.add,
                    )
                    fin2 = opool.tile([P, TD], BF16, tag="fin2")
                    nc.scalar.activation(fin2[:, :], fin[:, :],
                                         mybir.ActivationFunctionType.Identity,
                                         bias=neg1[:, :])
                    nc.sync.dma_start(out_d[wi * P:(wi + 1) * P, :], fin2[:, :])

    nc.compile()
    return nc


def host_prep(cfg: Config, x, Ws, As):
    import ml_dtypes as _md
    x = np.asarray(x, np.float32)
    Ws = np.asarray(Ws, np.float32)
    As = np.asarray(As, np.float32)
    n = x.shape[0]
    xt = np.zeros((IN_FEAT, cfg.n_pad), np.float32)
    xt[:, :n] = x.T
    xt = xt.astype(_md.bfloat16)
    # wcat[f, h*64+o] = Ws[h,o,f]
    wcat = Ws.transpose(2, 0, 1).reshape(IN_FEAT, TD).astype(_md.bfloat16)
    a_r = As[:, OUT:, 0]  # [H, O]
    war = np.einsum("hof,ho->fh", Ws, a_r).astype(_md.bfloat16)
    iota = np.tile(np.arange(P, dtype=np.float32), (P, 1))
    iota = iota.astype(_md.bfloat16)
    meta = cfg.meta_packed.astype(_md.bfloat16)
    in_maps = []
    for c in range(cfg.n_cores):
        in_maps.append({
            "xt": xt, "wcat": wcat, "war": war,
            "iota": np.ascontiguousarray(iota),
            "idx": np.ascontiguousarray(cfg.idx_packed[c]),
            "meta": np.ascontiguousarray(meta[c]),
        })
    return in_maps


from concourse.bass_utils import run_bass_kernel_spmd

LAST_EXEC_TIME_NS = None


def kernel(x, src, dst, Ws, As):
    """Full-input entry point: shards internally across 8 NeuronCores."""
    global LAST_EXEC_TIME_NS
    x = np.asarray(x, np.float32)
    src = np.asarray(src)
    dst = np.asarray(dst)
    Ws = np.asarray(Ws, np.float32)
    As = np.asarray(As, np.float32)
    n = x.shape[0]

    cfg = Config(n, src, dst, n_cores=8)
    nc = build_program(cfg)
    in_maps = host_prep(cfg, x, Ws, As)
    import os as _os
    _trace = _os.environ.get("KERNEL_TRACE", "0") == "1"
    _tdir = _os.environ.get("KERNEL_TRACE_DIR") or None
    res = run_bass_kernel_spmd(nc, in_maps, core_ids=list(range(cfg.n_cores)),
                               trace=_trace, tmpdir=_tdir)
    LAST_EXEC_TIME_NS = res.exec_time_ns
    out = np.concatenate([res.results[c]["out"] for c in range(cfg.n_cores)],
                         axis=0)[:n]
    return np.ascontiguousarray(out, dtype=np.float32)
